# revision 1
# baseline (speedup 1.0000x reference)
"""GAT layer (gnn_message_passing) Bass kernel for 8 Trainium2 NeuronCores.

Row-sharded: core c computes output rows [c*R, (c+1)*R) of
    out = softmax(mask(leakyrelu(s_src[i]+s_dst[j]), adj)) @ (h @ W.T)

Math notes:
  - e[i,j] = leakyrelu(a_src.Wh_i + a_dst.Wh_j, 0.2);  s_src = Wh@a_src = h@(W.T a_src)
  - softmax rewritten unnormalized: p = adj * exp(e)  (no max-subtract needed:
    |e| <= ~6 for this data scale, exp stays well inside fp32), out_i = (p @ Wh)_i / sum_j p[i,j]
  - masked entries are exactly 0 (reference uses -9e15 -> exp == 0).

Layout: everything on-device runs transposed, [j (source node) on partitions,
i (dest node) on free]. The host hands each core adj[own_rows].T so the mask
tiles stream j-major; p.T tiles then feed the TensorEngine directly as the
stationary operand for out = p @ [Wh | 1] with zero on-chip transposes.
"""

import functools
import sys

sys.path.insert(0, "/opt/trn_rl_repo")

import numpy as np

import bass_rust
import concourse.bass as bass
import concourse.mybir as mybir
import concourse.tile as tile
from concourse.masks import make_identity
from concourse.bass_utils import run_bass_kernel_spmd

F32 = mybir.dt.float32
I32 = mybir.dt.int32
AF = mybir.ActivationFunctionType
ALU = mybir.AluOpType

N_CORES = 8


def _patch_tail_drain():
    """This walrus build caps sync waits at 1 per instruction (2 for EVSEM),
    but Tile emits multi-wait instructions in two places: regular insts via
    assign_waits, and the tail drain. Split surplus waits onto same-engine
    wait-only NOPs placed immediately before (regular) / after (tail drain)
    the owning instruction."""
    from concourse.tile import ScopedClock, TileContext

    if getattr(TileContext, "_drain_patched", False):
        return

    _orig_loi = TileContext._lower_ordered_insts

    def _lower_ordered_insts(self, ordered):
        nc = self.nc
        ws_id = 0
        for bbname in list(ordered.keys()):
            insts = ordered[bbname]
            new = []
            for inst in insts:
                si = inst.sync_info
                if si is not None:
                    cap = 2 if isinstance(inst, mybir.InstEventSemaphore) else 1
                    waits = list(si.on_wait)
                    if len(waits) > cap:
                        extra, keep = waits[:-cap], waits[-cap:]
                        for w in extra:
                            nop = mybir.InstNoOp(
                                name=f"{inst.name}-ws{ws_id}", ins=[], outs=[]
                            )
                            ws_id += 1
                            nop.engine = inst.engine
                            nop.sync_info = bass_rust.SyncInfo(
                                on_wait=[w], on_update=[]
                            )
                            nc.register_instruction(nop, overwrite=True)
                            new.append(nop)
                        inst.sync_info = bass_rust.SyncInfo(
                            on_wait=keep, on_update=list(si.on_update)
                        )
                new.append(inst)
            ordered[bbname] = new
        return _orig_loi(self, ordered)

    TileContext._lower_ordered_insts = _lower_ordered_insts

    def _drain_and_barrier(self, tick_clock, wait_clock):
        drain_inst = self.nc.sync.drain()
        wait_clock.add_sem_waits(
            drain_inst.ins, ScopedClock({None: tick_clock.global_clock})
        )
        si = drain_inst.ins.sync_info
        if si is not None and len(si.on_wait) > 1:
            waits = list(si.on_wait)
            drain_inst.ins.sync_info = bass_rust.SyncInfo(
                on_wait=[waits[0]], on_update=list(si.on_update)
            )
            for w in waits[1:]:
                nop = self.nc.sync.nop(nofuse=True)
                nop.ins.sync_info = bass_rust.SyncInfo(on_wait=[w], on_update=[])
        self.nc.all_engine_barrier()
        assert self.sems is not None
        popped = self.nc._tile_sem_poison_stack.pop()
        assert popped is self._sem_poison
        self.nc.clear_and_free_semaphores(list(self.sems.allocated().values()))
        self.nc.all_engine_barrier()

    TileContext._drain_and_barrier = _drain_and_barrier
    TileContext._drain_patched = True

    # walrus is invoked with --enable-ldw-opt=false, which leaves every
    # LDWEIGHTS serialized against the previous matmul's drain (~2x matmul
    # cost for back-to-back weight-swapping streams). Re-enable it.
    import concourse.bass_utils as _bu

    _orig_run_command = _bu.run_command

    def _run_command(cmd, *a, **kw):
        cmd = [
            "--enable-ldw-opt=true" if c == "--enable-ldw-opt=false" else c
            for c in cmd
        ]
        return _orig_run_command(cmd, *a, **kw)

    _bu.run_command = _run_command


def build_gat_nc(N=8192, R=1024, FIN=256, FOUT=128):
    """Build the per-core Bass program (transposed layout). All cores run the
    same program on different data slices."""
    _patch_tail_drain()
    from concourse.tile_rust import add_dep_helper

    P = 128
    FK = FIN // P          # fin chunks (contraction for Wh)
    NCH = N // P           # 128-row j-chunks over all N source nodes
    RB = R // P            # 128-wide i-subblocks per core

    nc = bass.Bass()
    hT_t = nc.dram_tensor("hT", [FIN, N], F32, kind="ExternalInput")
    hTown_t = nc.dram_tensor("hT_own", [FIN, R], F32, kind="ExternalInput")
    adjT_t = nc.dram_tensor("adjT_blk", [N, R], I32, kind="ExternalInput")
    w_t = nc.dram_tensor("W", [FOUT, FIN], F32, kind="ExternalInput")
    wT_t = nc.dram_tensor("WT", [FIN, FOUT], F32, kind="ExternalInput")
    a_t = nc.dram_tensor("a", [2 * FOUT, 1], F32, kind="ExternalInput")
    out_t = nc.dram_tensor("out_blk", [R, FOUT], F32, kind="ExternalOutput")
    import os

    debug = bool(os.environ.get("GAT_DEBUG"))
    if debug:
        dbg_sums = nc.dram_tensor("dbg_sums", [1, R], F32, kind="ExternalOutput")
        dbg_outT = nc.dram_tensor("dbg_outT", [P, R], F32, kind="ExternalOutput")
        dbg_recip = nc.dram_tensor("dbg_recip", [P, R // P], F32, kind="ExternalOutput")

    with tile.TileContext(nc) as tc:
        with tc.tile_pool(name="persist", bufs=1) as persist:
            ident = persist.tile([P, P], F32)
            make_identity(nc, ident)
            ones_col = persist.tile([P, 1], F32)
            nc.vector.memset(ones_col, 1.0)
            ones_row = persist.tile([1, P], F32)
            nc.vector.memset(ones_row, 1.0)
            whs_sb = persist.tile([P, NCH, FOUT], F32)       # Wh, j on partitions
            sdst_col = persist.tile([P, NCH], F32)           # s_dst, partition-major
            ssrc_col = persist.tile([P, RB], F32)            # s_src own rows, partition-major
            ssrc_bcast = persist.tile([P, R], F32)           # s_src bcast to all partitions
            rhs_aug = persist.tile([P, FK, FOUT + 1], F32)   # [W.T | w_dst] per fin chunk
            wsrc_sb = persist.tile([P, FK], F32)             # w_src per fin chunk

            # ---------------- prologue: Wh, s_dst, s_src ----------------
            with (
                tc.tile_pool(name="pro1", bufs=1) as pro1,
                tc.tile_pool(name="pro_ps", bufs=2, space="PSUM") as pro_ps,
                tc.tile_pool(name="pro_ps1", bufs=1, space="PSUM") as pro_ps1,
            ):
                w_sb = pro1.tile([P, FIN], F32)
                nc.sync.dma_start(out=w_sb, in_=w_t[:, :])
                acol = pro1.tile([P, 2], F32)
                nc.sync.dma_start(out=acol[:, 0:1], in_=a_t[0:FOUT, :])       # a_src
                nc.sync.dma_start(out=acol[:, 1:2], in_=a_t[FOUT : 2 * FOUT, :])  # a_dst
                # hT staged whole: [fin, N] as FK tiles of [128, N]
                hT_sb = pro1.tile([P, FK, N], F32)
                for k in range(FK):
                    nc.sync.dma_start(
                        out=hT_sb[:, k, :], in_=hT_t[k * P : (k + 1) * P, :]
                    )
                hTo_sb = pro1.tile([P, FK, R], F32)
                for k in range(FK):
                    nc.sync.dma_start(
                        out=hTo_sb[:, k, :], in_=hTown_t[k * P : (k + 1) * P, :]
                    )

                for k in range(FK):
                    nc.sync.dma_start(
                        out=rhs_aug[:, k, 0:FOUT],
                        in_=wT_t[k * P : (k + 1) * P, :],
                    )
                    wchunk = w_sb[:, k * P : (k + 1) * P]
                    pw = pro_ps1.tile([P, 2], F32, tag="wv")
                    nc.tensor.matmul(pw[:, 0:1], wchunk, acol[:, 1:2], start=True, stop=True)
                    nc.tensor.matmul(pw[:, 1:2], wchunk, acol[:, 0:1], start=True, stop=True)
                    nc.vector.tensor_copy(out=rhs_aug[:, k, FOUT : FOUT + 1], in_=pw[:, 0:1])
                    nc.vector.tensor_copy(out=wsrc_sb[:, k : k + 1], in_=pw[:, 1:2])

                # Wh + s_dst for all N source nodes
                for c in range(NCH):
                    wh_ps = pro_ps.tile([P, FOUT + 1], F32, tag="wh")
                    for k in range(FK):
                        nc.tensor.matmul(
                            wh_ps,
                            hT_sb[:, k, c * P : (c + 1) * P],
                            rhs_aug[:, k, :],
                            start=(k == 0),
                            stop=(k == FK - 1),
                        )
                    nc.vector.tensor_copy(out=whs_sb[:, c, :], in_=wh_ps[:, 0:FOUT])
                    nc.vector.tensor_copy(out=sdst_col[:, c : c + 1], in_=wh_ps[:, FOUT : FOUT + 1])

                # s_src for own rows
                for b in range(RB):
                    sp = pro_ps1.tile([P, 1], F32, tag="ss")
                    for k in range(FK):
                        nc.tensor.matmul(
                            sp,
                            hTo_sb[:, k, b * P : (b + 1) * P],
                            wsrc_sb[:, k : k + 1],
                            start=(k == 0),
                            stop=(k == FK - 1),
                        )
                    nc.vector.tensor_copy(out=ssrc_col[:, b : b + 1], in_=sp)

                # s_src broadcast across partitions, all on-chip: transpose
                # the per-partition columns into one row, then outer-product
                # with a ones column (K=1 matmul) to replicate it down the
                # partition dim.
                srow_ps = pro_ps1.tile([1, R], F32, tag="srow")
                for b in range(RB):
                    nc.tensor.transpose(
                        srow_ps[:, b * P : (b + 1) * P], ssrc_col[:, b : b + 1], ident
                    )
                srow_sb = pro1.tile([1, R], F32)
                nc.vector.tensor_copy(out=srow_sb, in_=srow_ps)
                sbc_ps = pro_ps1.tile([P, R], F32, tag="sbc")
                BSEG = 512 if R % 512 == 0 else R
                for s in range(R // BSEG):
                    nc.tensor.matmul(
                        sbc_ps[:, s * BSEG : (s + 1) * BSEG],
                        ones_row,
                        srow_sb[:, s * BSEG : (s + 1) * BSEG],
                        start=True,
                        stop=True,
                    )
                nc.vector.tensor_copy(out=ssrc_bcast, in_=sbc_ps)

            # ------------- main loop over j-chunks (transposed layout) -------------
            # out.T accumulates in PSUM: for each j-chunk, Wh[jc] is the
            # stationary operand (one LDWEIGHTS) and p.T streams through as
            # wide 512-col moving operands; a ones-column stationary gives the
            # softmax denominators the same way.
            SEG = 512 if R % 512 == 0 else R
            NSEG = R // SEG
            EB = 4 if NCH % 4 == 0 else 1   # Exp batch: chunks per ACTIVATE
            with (
                tc.tile_pool(name="adjp", bufs=4) as adjp,
                tc.tile_pool(name="ep", bufs=2) as ep,
                tc.tile_pool(name="xp", bufs=2) as xp,
                tc.tile_pool(name="pp", bufs=4) as pp,
                tc.tile_pool(name="sm", bufs=2) as sm,
                tc.tile_pool(name="osb", bufs=2) as osb,
                tc.tile_pool(name="out_ps", bufs=1, space="PSUM") as out_ps,
                tc.tile_pool(name="tr_ps", bufs=2, space="PSUM") as tr_ps,
            ):
                psum_outT = [
                    out_ps.tile([P, SEG], F32, tag=f"poT{s}", name=f"poT{s}")
                    for s in range(NSEG)
                ]
                psum_sums = [
                    out_ps.tile([1, SEG], F32, tag=f"psm{s}", name=f"psm{s}")
                    for s in range(NSEG)
                ]
                eT_g = None
                expT_g = None
                for jc in range(NCH):
                    g = jc % EB
                    if g == 0:
                        eT_g = ep.tile([P, EB, R], F32, tag="e", name="eT_g")
                    nc.scalar.activation(
                        out=eT_g[:, g, :],
                        in_=ssrc_bcast,
                        func=AF.Prelu,
                        bias=sdst_col[:, jc : jc + 1],
                        scale=1.0,
                        alpha=0.2,
                    )
                    if g == EB - 1:
                        expT_g = xp.tile([P, EB, R], F32, tag="x", name="expT_g")
                        nc.scalar.activation(out=expT_g, in_=eT_g, func=AF.Exp)
                    else:
                        continue
                    for gg in range(EB):
                        jcc = jc - (EB - 1) + gg
                        adjT_ch = adjp.tile([P, R], I32, tag="adj", name="adjT_ch")
                        nc.sync.dma_start(
                            out=adjT_ch, in_=adjT_t[jcc * P : (jcc + 1) * P, :]
                        )
                        pT_ch = pp.tile([P, R], F32, tag="p", name="pT_ch")
                        nc.gpsimd.memset(pT_ch, 0.0)
                        nc.vector.copy_predicated(
                            out=pT_ch, mask=adjT_ch, data=expT_g[:, gg, :]
                        )
                        for s in range(NSEG):
                            seg = pT_ch[:, s * SEG : (s + 1) * SEG]
                            nc.tensor.matmul(
                                psum_outT[s],
                                whs_sb[:, jcc, :],
                                seg,
                                start=(jcc == 0),
                                stop=(jcc == NCH - 1),
                            )
                            nc.tensor.matmul(
                                psum_sums[s],
                                ones_col,
                                seg,
                                start=(jcc == 0),
                                stop=(jcc == NCH - 1),
                            )

                # tail: denominators back to per-partition layout, transpose
                # out.T blocks, scale, store.
                sums_sb = sm.tile([1, R], F32, tag="ssb", name="sums_sb")
                for s in range(NSEG):
                    nc.vector.tensor_copy(
                        out=sums_sb[:, s * SEG : (s + 1) * SEG], in_=psum_sums[s]
                    )
                # [1, R] row -> [P, RB] per-partition columns via tiny PE
                # transposes ([1,128].T @ [[1]] = [128,1]).
                rsums_ps = tr_ps.tile([P, RB], F32, tag="rs", name="rsums_ps")
                for b in range(RB):
                    nc.tensor.transpose(
                        rsums_ps[:, b : b + 1],
                        sums_sb[0:1, b * P : (b + 1) * P],
                        ident[0:1, 0:1],
                    )
                recip_col = sm.tile([P, RB], F32, tag="rcc", name="recip_col")
                nc.vector.reciprocal(recip_col, rsums_ps)
                outT_sb = sm.tile([P, R], F32, tag="oT", name="outT_sb")
                for s in range(NSEG):
                    nc.vector.tensor_copy(
                        out=outT_sb[:, s * SEG : (s + 1) * SEG], in_=psum_outT[s]
                    )
                if debug:
                    nc.sync.dma_start(out=dbg_sums[:, :], in_=sums_sb)
                    nc.sync.dma_start(out=dbg_outT[:, :], in_=outT_sb)
                    nc.sync.dma_start(out=dbg_recip[:, :], in_=recip_col)
                for b in range(RB):
                    tr = tr_ps.tile([P, P], F32, tag="tr", name="tr")
                    nc.tensor.transpose(
                        tr, outT_sb[:, b * P : (b + 1) * P], ident
                    )
                    out_sb = osb.tile([P, FOUT], F32, tag="ob", name="out_sb")
                    nc.scalar.activation(
                        out=out_sb,
                        in_=tr,
                        func=AF.Copy,
                        bias=0.0,
                        scale=recip_col[:, b : b + 1],
                    )
                    nc.sync.dma_start(out=out_t[b * P : (b + 1) * P, :], in_=out_sb)

    return nc


@functools.lru_cache(maxsize=2)
def _compiled(N, R, FIN, FOUT):
    return build_gat_nc(N=N, R=R, FIN=FIN, FOUT=FOUT)


def run_gat(h, adj, W, a, trace=False, tmpdir=None):
    N, FIN = h.shape
    FOUT = W.shape[0]
    R = N // N_CORES
    nc = _compiled(N, R, FIN, FOUT)
    h = np.asarray(h, dtype=np.float32)
    adj = np.asarray(adj, dtype=np.int32)
    hT = np.ascontiguousarray(h.T)
    in_maps = []
    for c in range(N_CORES):
        sl = slice(c * R, (c + 1) * R)
        in_maps.append(
            {
                "hT": hT,
                "hT_own": np.ascontiguousarray(h[sl].T),
                "adjT_blk": np.ascontiguousarray(adj[sl].T),
                "W": np.ascontiguousarray(W, dtype=np.float32),
                "WT": np.ascontiguousarray(np.asarray(W, dtype=np.float32).T),
                "a": np.ascontiguousarray(
                    np.asarray(a, dtype=np.float32).reshape(2 * FOUT, 1)
                ),
            }
        )
    res = run_bass_kernel_spmd(
        nc, in_maps, core_ids=list(range(N_CORES)), trace=trace, tmpdir=tmpdir
    )
    out = np.concatenate([r["out_blk"] for r in res.results], axis=0)
    return out, res


def kernel(h, adj, W, a):
    out, _ = run_gat(np.asarray(h), np.asarray(adj), np.asarray(W), np.asarray(a))
    return out.astype(np.float32)

